# revision 14
# baseline (speedup 1.0000x reference)
"""BitFeedForward (BitNet b1.58 MLP) Trainium2 kernel, v2.

Full computation:
    h = gelu(bitlinear(x, w1, b1));  out = bitlinear(h, w2, b2)
    bitlinear(x,w,b) = actquant(rmsnorm(x)) @ ternary(w).T + b

Sharding: pure data-parallel over the 16384 tokens -> 2048 tokens/core on
8 NeuronCores.  No collectives.  Weights arrive host-pre-transposed
([K, out] layout; pure data movement) and are quantized on device.

Key numerics (same as v1):
  - rmsnorm factor cancels inside activation_quant's round(); the rsqrt
    chain only feeds the post-matmul dequant scale gamma = amax*r*mean|w|/127
  - quantized activations / ternary weights are exact in bf16; matmuls
    run bf16 x bf16 with f32 PSUM accumulation -> integer-exact
  - round() = fl(c*x + 1.5*2^23) - 1.5*2^23 (RNE); ACT table stays on the
    gelu set (Copy/Abs/Gelu only)

v2 structural changes vs v1 (802us -> target ~560us):
  - two-stream weight phase: w1/w2 are each read twice from HBM (stats
    stream, then quant stream) instead of being staged in a 128KB SBUF
    pool.  The quant stream is c-major so mm1 of tile 0 can start after
    the first 8 chunks: PE starts at ~55us instead of ~180us.
  - per-token biases are folded into the matmuls as a K=1 accumulation
    row (stationary = 1/gamma per token, moving = bias row), so the PSUM
    dequant is a single ACT op: h = Gelu(psum * gamma) for mm1 and
    out = Copy(psum * gamma2) for mm2.  No DVE pass over PSUM at all;
    the psum bank is recycled by exactly one engine (ACT), which removes
    the mm1-start stalls the v1 trace showed on S[165].
  - amax(h) via a tensor_max pair-tree (gelu output is >= -0.17, so the
    row max equals the abs-max with probability 1 up to 2^-4096) instead
    of eight 700ns DVE abs-max reduces.
  - k2 rounding moved to the idle GpSimd engine; x/out DMAs moved off
    the Sync queue (x on SP only in A; out on gpsimd) so SP carries only
    the dma transposes.
  - 1/gamma rows ([1,128] stationaries) produced by a tiny DRAM
    round-trip with an f32->bf16 cast on the SWDGE leg, issued ~3 tiles
    ahead of use.
  - HAM warmup/filler matmuls keep the PE clock at 8/8 through the
    w1-chase gaps of tile 0.
"""

import sys

for _p in ("/opt/trn_rl_repo",):
    if _p not in sys.path:
        sys.path.insert(0, _p)

from contextlib import ExitStack

import numpy as np

import concourse.bass as bass
import concourse.mybir as mybir
import concourse.tile as tile
from concourse.bass import ts

F32 = mybir.dt.float32
BF16 = mybir.dt.bfloat16
AF = mybir.ActivationFunctionType
ALU = mybir.AluOpType
AX = mybir.AxisListType.X
AXY = mybir.AxisListType.XY

P = 128
DIM = 1024
INNER = 4096
N_CORES = 8
TOKENS = 4 * 4096
TOK_PER_CORE = TOKENS // N_CORES  # 2048
TT = TOK_PER_CORE // P  # 16 token tiles per core
CC = 1024
LAG = 2  # mm2(t-LAG) interleaves with mm1(t)

MMAGIC = 12582912.0  # 1.5 * 2**23 : RNE rounding magic for |x| < 2^22
QB = 127.0
EPS = 1e-5

_DONE = object()


def _split_dma_waits(nc):
    """walrus codegen only supports ONE sync wait on DMA pseudo-instructions.
    Move all but one wait onto standalone EventSemaphore (add 0) instructions
    on the issuing engine -- semantically identical, codegen-legal."""
    idc = 0
    for f in nc.m.functions:
        for bb in f.blocks:
            changed = False
            new = []
            for inst in bb.instructions:
                tn = type(inst).__name__
                si = inst.sync_info
                if (
                    tn != "InstEventSemaphore"
                    and si is not None
                    and len(si.on_wait) > 1
                ):
                    waits = list(si.on_wait)
                    for w in waits[:-1]:
                        idc += 1
                        e = mybir.InstEventSemaphore(
                            name=f"WSPLIT-{idc}",
                            sync_type="semaphore",
                            id=w.id,
                            update_mode="sem-add-imm",
                            update_value=0,
                        )
                        e.engine = inst.engine
                        e.sync_info = mybir.SyncInfo(on_wait=[w], on_update=[])
                        nc.register_instruction(e, overwrite=True)
                        new.append(e)
                    inst.sync_info = mybir.SyncInfo(
                        on_wait=[waits[-1]], on_update=list(si.on_update)
                    )
                    changed = True
                new.append(inst)
            if changed:
                bb.instructions = new


def build(nc: bass.Bass, n_ttiles: int = TT, af_act=None):
    if af_act is None:
        af_act = AF.Gelu
    toks = n_ttiles * P
    lag = min(LAG, max(1, n_ttiles - 1))
    x_d = nc.dram_tensor("x", [toks, DIM], F32, kind="ExternalInput")
    # weights arrive pre-transposed from the host: [K, out]
    w1_d = nc.dram_tensor("w1t", [DIM, INNER], F32, kind="ExternalInput")
    b1_d = nc.dram_tensor("b1", [INNER], F32, kind="ExternalInput")
    w2_d = nc.dram_tensor("w2t", [INNER, DIM], F32, kind="ExternalInput")
    b2_d = nc.dram_tensor("b2", [DIM], F32, kind="ExternalInput")
    out_d = nc.dram_tensor("out", [toks, DIM], F32, kind="ExternalOutput")

    with tile.TileContext(nc) as tc, ExitStack() as ctx:
        consts = ctx.enter_context(tc.tile_pool(name="consts", bufs=1))
        dram = ctx.enter_context(tc.tile_pool(name="dram", bufs=1, space="DRAM"))
        gdram = ctx.enter_context(tc.tile_pool(name="gdram", bufs=6, space="DRAM"))
        psA = ctx.enter_context(tc.tile_pool(name="psA", bufs=5, space="PSUM"))
        psB = ctx.enter_context(tc.tile_pool(name="psB", bufs=2, space="PSUM"))
        psS = ctx.enter_context(tc.tile_pool(name="psS", bufs=1, space="PSUM"))
        st_p = ctx.enter_context(tc.tile_pool(name="st", bufs=2))
        g_p = ctx.enter_context(tc.tile_pool(name="g", bufs=lag + 1 if lag < 2 else 3))
        gr_p = ctx.enter_context(tc.tile_pool(name="gr", bufs=lag + 1 if lag < 2 else 3))

        ones = consts.tile([P, 1], F32)
        nc.vector.memset(ones, 1.0)
        ones_bf = consts.tile([P, 1], BF16)
        nc.vector.memset(ones_bf, 1.0)
        # w1Tc[c][:, j, m]: K-slice j (128 rows), inner cols c*1024 + m
        w1Tc = [
            consts.tile([P, 8, CC], BF16, tag=f"w1T{c}", name=f"w1T{c}")
            for c in range(4)
        ]
        # w2T[:, r, m]: K-slice r of 32 (128 rows of w2t), out m (1024)
        w2T = consts.tile([P, 32, DIM], BF16, tag="w2T")
        # bcast scalars: 0=ws1 1=mwd1 2=ws2 3=mwd2 4=127*ws1 5=127*ws2
        scal = consts.tile([P, 6], F32, tag="scal")
        b1row = consts.tile([1, INNER], BF16, tag="b1row")
        b2row = consts.tile([1, DIM], BF16, tag="b2row")
        dsc = dram.tile([1, 6], F32)

        wstg_p = ctx.enter_context(tc.tile_pool(name="wstg", bufs=2))
        xin_p = ctx.enter_context(tc.tile_pool(name="xin", bufs=1))
        k1q_p = ctx.enter_context(tc.tile_pool(name="k1q", bufs=1))
        k1T_p = ctx.enter_context(tc.tile_pool(name="k1T", bufs=3))
        k2m_p = ctx.enter_context(tc.tile_pool(name="k2m", bufs=1))
        qb_p = ctx.enter_context(tc.tile_pool(name="qb", bufs=2))
        k2T_p = ctx.enter_context(tc.tile_pool(name="k2T", bufs=lag + 1))
        h_p = ctx.enter_context(tc.tile_pool(name="h", bufs=4))
        mt_p = ctx.enter_context(tc.tile_pool(name="mt", bufs=1))
        out_p = ctx.enter_context(tc.tile_pool(name="out", bufs=1))

        # ---------------- helpers ----------------
        def _bcast(dram_sc, sb_dst, src):
            nc.sync.dma_start(dram_sc, src)
            nc.sync.dma_start(sb_dst, dram_sc.to_broadcast(list(sb_dst.shape)))

        def _rsqrt_newton(v, seed, iters, tg):
            r = st_p.tile([P, 1], F32, tag=f"rs_r{tg}")
            nc.vector.memset(r, seed)
            for _ in range(iters):
                rr = st_p.tile([P, 1], F32, tag=f"rs_rr{tg}")
                nc.vector.tensor_mul(rr, r, r)
                t = st_p.tile([P, 1], F32, tag=f"rs_t{tg}")
                nc.vector.scalar_tensor_tensor(
                    out=t, in0=rr, scalar=-0.5, in1=v, op0=ALU.mult, op1=ALU.mult
                )
                r2 = st_p.tile([P, 1], F32, tag=f"rs_r2{tg}")
                nc.vector.scalar_tensor_tensor(
                    out=r2, in0=t, scalar=1.5, in1=r, op0=ALU.add, op1=ALU.mult
                )
                r = r2
            return r

        def _chain_pre(amax, mv, seed, iters, tg):
            """c = 127/max(amax,eps); plus am, rec, r=rsqrt(var+mean^2+eps),
            sq=sqrt(var+mean^2+eps) for the scal-dependent tail."""
            am = g_p.tile([P, 1], F32, tag=f"c_am{tg}")
            nc.vector.tensor_scalar(
                out=am, in0=amax, scalar1=EPS, scalar2=None, op0=ALU.max
            )
            rec = g_p.tile([P, 1], F32, tag=f"c_rec{tg}")
            nc.vector.reciprocal(rec, am)
            c = st_p.tile([P, 1], F32, tag=f"c_c{tg}")
            nc.vector.tensor_scalar(
                out=c, in0=rec, scalar1=QB, scalar2=None, op0=ALU.mult
            )
            v = st_p.tile([P, 1], F32, tag=f"g_v{tg}")
            nc.vector.tensor_scalar(
                out=v, in0=mv[:, 0:1], scalar1=mv[:, 0:1], scalar2=None, op0=ALU.mult
            )
            nc.vector.tensor_scalar(
                out=v, in0=v, scalar1=mv[:, 1:2], scalar2=EPS, op0=ALU.add, op1=ALU.add
            )
            r = _rsqrt_newton(v, seed, iters, tg)
            rk = g_p.tile([P, 1], F32, tag=f"g_r{tg}")
            nc.vector.tensor_copy(rk, r)
            sq = g_p.tile([P, 1], F32, tag=f"g_sq{tg}")
            nc.vector.tensor_mul(sq, v, r)  # sqrt(v) = v * rsqrt(v)
            return c, (am, rec, rk, sq)

        def _chain_post(pre, mwd_col, qws_col, tg):
            """g = am*mwd*r ; ginv = rec*qws*sq ; row = [1,P] bf16 of ginv"""
            am, rec, r, sq = pre
            g = g_p.tile([P, 1], F32, tag=f"g{tg}")
            nc.vector.scalar_tensor_tensor(
                out=g,
                in0=am,
                scalar=scal[:, mwd_col : mwd_col + 1],
                in1=r,
                op0=ALU.mult,
                op1=ALU.mult,
            )
            ginv = st_p.tile([P, 1], F32, tag=f"gi{tg}")
            nc.vector.scalar_tensor_tensor(
                out=ginv,
                in0=rec,
                scalar=scal[:, qws_col : qws_col + 1],
                in1=sq,
                op0=ALU.mult,
                op1=ALU.mult,
            )
            row = _ginv_row(ginv, tg)
            return g, row

        def _ginv_row(ginv, tg):
            """[P,1] f32 -> [1,P] bf16 via DRAM round-trip (cast on SWDGE)."""
            gd = gdram.tile([P, 1], F32, tag=f"gd{tg}")
            nc.gpsimd.dma_start(gd, ginv)
            row = gr_p.tile([1, P], BF16, tag=f"grow{tg}")
            nc.gpsimd.dma_start(row, gd.rearrange("p o -> o p"))
            return row

        _PARTS = {}

        def wscale(tg, n_elems, ws_col, mwd_col, qws_col, dsc_off):
            partials = _PARTS[tg]
            psum_v = st_p.tile([P, 1], F32, tag=f"psumv{tg}")
            nc.vector.tensor_reduce(out=psum_v, in_=partials, axis=AX, op=ALU.add)
            tot = psS.tile([1, 1], F32, tag="tot")
            nc.tensor.matmul(tot, psum_v, ones[:, 0:1], start=True, stop=True)
            mean = st_p.tile([1, 1], F32, tag=f"mean{tg}")
            nc.scalar.activation(mean, tot, AF.Copy, bias=0.0, scale=1.0 / n_elems)
            mw = st_p.tile([1, 1], F32, tag=f"mw{tg}")
            nc.vector.tensor_scalar(
                out=mw, in0=mean, scalar1=EPS, scalar2=None, op0=ALU.max
            )
            wsv = st_p.tile([1, 1], F32, tag=f"wsv{tg}")
            nc.vector.reciprocal(wsv, mw)
            mwd = st_p.tile([1, 1], F32, tag=f"mwd{tg}")
            nc.vector.tensor_scalar(
                out=mwd, in0=mw, scalar1=1.0 / QB, scalar2=None, op0=ALU.mult
            )
            qws = st_p.tile([1, 1], F32, tag=f"qws{tg}")
            nc.vector.tensor_scalar(
                out=qws, in0=wsv, scalar1=QB, scalar2=None, op0=ALU.mult
            )
            _bcast(dsc[0:1, dsc_off : dsc_off + 1], scal[:, ws_col : ws_col + 1], wsv)
            _bcast(
                dsc[0:1, dsc_off + 1 : dsc_off + 2],
                scal[:, mwd_col : mwd_col + 1],
                mwd,
            )
            _bcast(
                dsc[0:1, dsc_off + 4 : dsc_off + 5],
                scal[:, qws_col : qws_col + 1],
                qws,
            )

        def quant_chunk(wf, dst, ws_b, parity):
            """round(wf*ws) -> dst (bf16, clipped to [-1,1]).  wf is consumed."""
            nc.gpsimd.tensor_scalar(
                out=wf,
                in0=wf,
                scalar1=ws_b,
                scalar2=MMAGIC,
                op0=ALU.mult,
                op1=ALU.add,
            )
            if parity == 0:
                nc.scalar.activation(dst, wf, AF.Copy, bias=-MMAGIC, scale=1.0)
            else:
                nc.vector.tensor_scalar(
                    out=dst, in0=wf, scalar1=MMAGIC, scalar2=None, op0=ALU.subtract
                )
            nc.vector.tensor_scalar(
                out=dst, in0=dst, scalar1=-1.0, scalar2=1.0, op0=ALU.max, op1=ALU.min
            )

        # ---------------- token-loop stages ----------------
        A = {}

        def emit_A(t):
            xin = xin_p.tile([P, DIM], F32, tag="x")
            nc.sync.dma_start(xin, x_d[ts(t, P), :])
            amax = st_p.tile([P, 1], F32, tag="amax")
            nc.vector.tensor_reduce(
                out=amax, in_=xin, axis=AX, op=ALU.max, apply_absolute_value=True
            )
            stat6 = st_p.tile([P, 2, 6], F32, tag="st6")
            xv = xin.rearrange("p (a b) -> p a b", a=2)
            for a in range(2):
                nc.vector.bn_stats(out=stat6[:, a, :], in_=xv[:, a, :])
            mv = st_p.tile([P, 2], F32, tag="mv")
            nc.vector.bn_aggr(out=mv, in_=stat6)
            c1, pre1 = _chain_pre(amax, mv, 1.0, 4, "1")
            k1 = k1q_p.tile([P, DIM], BF16, tag="k1q")
            for q in range(2):
                k1m = k2m_p.tile([P, 512], F32, tag="k2m", name=f"k1m{q}")
                nc.scalar.activation(
                    k1m, xin[:, ts(q, 512)], AF.Copy, bias=MMAGIC, scale=c1
                )
                nc.vector.tensor_scalar(
                    out=k1[:, ts(q, 512)],
                    in0=k1m,
                    scalar1=MMAGIC,
                    scalar2=None,
                    op0=ALU.subtract,
                )
            k1T = k1T_p.tile([P, 8, P], BF16, tag="k1T")
            nc.sync.dma_start_transpose(k1T, k1)
            A[t] = [k1T, pre1, None, None]

        def emit_A_post(t):
            g1, g1row = _chain_post(A[t][1], 1, 4, "1")
            A[t][2] = g1
            A[t][3] = g1row

        def emit_B_slice(t, n):
            k1T, _, g1, g1row = A[t]
            ps = psA.tile([P, 512], F32, tag="ps1")
            nc.tensor.matmul(
                ps, g1row, b1row[0:1, ts(n, 512)], start=True, stop=False
            )
            for j in range(8):
                nc.tensor.matmul(
                    ps,
                    k1T[:, j, :],
                    w1Tc[n // 2][:, j, (n % 2) * 512 : (n % 2) * 512 + 512],
                    start=False,
                    stop=(j == 7),
                )
            h = _HCH[t % 2][n // 2][:, (n % 2) * 512 : (n % 2) * 512 + 512]
            nc.scalar.activation(h, ps, af_act, bias=0.0, scale=g1)
            nc.vector.bn_stats(out=_HST[t % 2][:, n, :], in_=h)

        _HCH = {0: None, 1: None}
        _HST = {0: None, 1: None}

        def begin_B(t):
            _HCH[t % 2] = [
                h_p.tile([P, CC], BF16, tag="h", name=f"h{t}_{i}") for i in range(4)
            ]
            _HST[t % 2] = st_p.tile(
                [P, 8, 6], F32, tag=f"hst{t % 2}", name=f"hst{t}"
            )

        D = {}

        def emit_C(t):
            hch = _HCH[t % 2]
            m = mt_p.tile([P, 512], BF16, tag="mt")
            nc.vector.tensor_max(m, hch[0][:, 0:512], hch[0][:, 512:1024])
            for cc in range(1, 4):
                nc.vector.tensor_max(m, m, hch[cc][:, 0:512])
                nc.vector.tensor_max(m, m, hch[cc][:, 512:1024])
            amaxh = st_p.tile([P, 1], F32, tag="amaxh")
            nc.vector.tensor_reduce(out=amaxh, in_=m, axis=AX, op=ALU.max)
            mvh = st_p.tile([P, 2], F32, tag="mvh")
            nc.vector.bn_aggr(out=mvh, in_=_HST[t % 2])
            c2, pre2 = _chain_pre(amaxh, mvh, 1.75, 4, "2")
            k2T = k2T_p.tile([P, 4, 8, P], BF16, tag="k2T")
            for cc in range(4):
                k2c = qb_p.tile([P, CC], BF16, tag="qb", name=f"k2c{cc}")
                for q in range(2):
                    k2m = k2m_p.tile([P, 512], F32, tag="k2m", name=f"k2m{q}")
                    nc.gpsimd.tensor_scalar(
                        out=k2m,
                        in0=hch[cc][:, ts(q, 512)],
                        scalar1=c2,
                        scalar2=MMAGIC,
                        op0=ALU.mult,
                        op1=ALU.add,
                    )
                    nc.scalar.activation(
                        k2c[:, ts(q, 512)], k2m, AF.Copy, bias=-MMAGIC, scale=1.0
                    )
                nc.sync.dma_start_transpose(k2T[:, cc, :, :], k2c)
            D[t] = [k2T, pre2, None, None]

        def emit_C_post(t):
            g2, g2row = _chain_post(D[t][1], 3, 5, "2")
            D[t][2] = g2
            D[t][3] = g2row

        def emit_D(t):
            k2T, _, g2, g2row = D.pop(t)
            ot = out_p.tile([P, DIM], F32, tag="ot")
            for n in range(2):
                ps2 = psB.tile([P, 512], F32, tag="ps2")
                nc.tensor.matmul(
                    ps2, g2row, b2row[0:1, ts(n, 512)], start=True, stop=False
                )
                for cc in range(4):
                    for j in range(8):
                        nc.tensor.matmul(
                            ps2,
                            k2T[:, cc, j, :],
                            w2T[:, 8 * cc + j, ts(n, 512)],
                            start=False,
                            stop=(cc == 3 and j == 7),
                        )
                nc.scalar.activation(
                    ot[:, ts(n, 512)], ps2, AF.Copy, bias=0.0, scale=g2
                )
            nc.gpsimd.dma_start(out_d[ts(t, P), :], ot)

        def warm_pe(src, k):
            for _ in range(k):
                pw = psA.tile([P, 512], F32, tag="ps1")
                nc.tensor.matmul(pw[0:1, :], ones_bf, src, start=True, stop=True)

        # ---------------- W phase ----------------
        # w1 stats stream (HBM pass 1)
        part1 = st_p.tile([P, 32], F32, tag="partw1")
        _PARTS["w1"] = part1
        for c in range(4):
            for j in range(8):
                wf = wstg_p.tile([P, CC], F32, tag="wstg")
                nc.gpsimd.dma_start(wf, w1_d[ts(j, P), ts(c, CC)])
                nc.vector.tensor_reduce(
                    out=part1[:, c * 8 + j : c * 8 + j + 1],
                    in_=wf,
                    axis=AX,
                    op=ALU.add,
                    apply_absolute_value=True,
                )
        emit_A(0)
        emit_A(1)
        wscale("w1", 32 * P * 1024, 0, 1, 4, 0)
        ws1_b = scal[:, 0:1]
        emit_A_post(0)
        emit_A_post(1)
        emit_A(2)
        emit_A_post(2)
        # bias rows (f32 -> bf16 cast on the SWDGE leg)
        nc.gpsimd.dma_start(b1row, b1_d.rearrange("(a c) -> a c", a=1))
        nc.gpsimd.dma_start(b2row, b2_d.rearrange("(a c) -> a c", a=1))

        # w1 quant stream (HBM pass 2), c-major so mm1 chunks unlock in order
        begin_B(0)
        for c in range(4):
            for j in range(8):
                wf = wstg_p.tile([P, CC], F32, tag="wstg")
                nc.gpsimd.dma_start(wf, w1_d[ts(j, P), ts(c, CC)])
                quant_chunk(wf, w1Tc[c][:, j, :], ws1_b, j % 2)
            if c == 0:
                warm_pe(w1Tc[0][:, 0, 0:512], 12)
            emit_B_slice(0, 2 * c)
            emit_B_slice(0, 2 * c + 1)
            if c < 3:
                warm_pe(w1Tc[c][:, 7, 0:512], 8)
        emit_C(0)

        # w2: stats stream + scale + quant stream, interleaved into tiles 1..3
        def w2gen():
            part2 = st_p.tile([P, 32], F32, tag="partw2")
            _PARTS["w2"] = part2
            for r in range(32):
                wf = wstg_p.tile([P, CC], F32, tag="wstg")
                nc.scalar.dma_start(wf, w2_d[ts(r, P), :])
                if r % 2 == 0:
                    nc.vector.tensor_reduce(
                        out=part2[:, r : r + 1],
                        in_=wf,
                        axis=AX,
                        op=ALU.add,
                        apply_absolute_value=True,
                    )
                else:
                    nc.scalar.activation(
                        wf,
                        wf,
                        AF.Abs,
                        bias=0.0,
                        scale=1.0,
                        accum_out=part2[:, r : r + 1],
                    )
                yield
            wscale("w2", 32 * P * DIM, 2, 3, 5, 1)
            yield
            ws2_b = scal[:, 2:3]
            for r in range(32):
                wf = wstg_p.tile([P, CC], F32, tag="wstg")
                nc.scalar.dma_start(wf, w2_d[ts(r, P), :])
                quant_chunk(wf, w2T[:, r, :], ws2_b, r % 2)
                yield

        gens = {"g": w2gen(), "alive": True}

        def il_step(k=1):
            for _ in range(k):
                if gens["alive"]:
                    gens["alive"] = next(gens["g"], _DONE) is not _DONE

        # ---------------- token loop ----------------
        for t in range(1, n_ttiles):
            begin_B(t)
            for n in range(8):
                emit_B_slice(t, n)
                il_step(5)
            if t + 2 < n_ttiles:
                emit_A(t + 2)
                emit_A_post(t + 2)
            emit_C(t)
            if t == lag:
                while gens["alive"]:
                    il_step()
                for u in range(lag + 1):
                    emit_C_post(u)
            elif t > lag:
                emit_C_post(t)
            if t >= lag:
                emit_D(t - lag)
        while gens["alive"]:
            il_step()
        for u in range(max(1, n_ttiles - lag), n_ttiles):
            if D[u][2] is None:
                emit_C_post(u)
        for t in range(n_ttiles - lag, n_ttiles):
            emit_D(t)

    _split_dma_waits(nc)
    return nc, x_d, out_d


_CACHE = {}


def _get_compiled(n_ttiles=TT):
    if n_ttiles not in _CACHE:
        nc = bass.Bass()
        build(nc, n_ttiles)
        nc.finalize()
        _CACHE[n_ttiles] = nc
    return _CACHE[n_ttiles]


def kernel(x, w1, b1, w2, b2, _trace=False, _tmpdir=None):
    from concourse import bass_utils

    nc = _get_compiled(TT)
    xf = np.ascontiguousarray(x.reshape(TOKENS, DIM).astype(np.float32))
    w1t = np.ascontiguousarray(w1.astype(np.float32).T)  # [1024, 4096]
    b1 = np.ascontiguousarray(b1.astype(np.float32))
    w2t = np.ascontiguousarray(w2.astype(np.float32).T)  # [4096, 1024]
    b2 = np.ascontiguousarray(b2.astype(np.float32))
    in_maps = [
        {
            "x": xf[c * TOK_PER_CORE : (c + 1) * TOK_PER_CORE],
            "w1t": w1t,
            "b1": b1,
            "w2t": w2t,
            "b2": b2,
        }
        for c in range(N_CORES)
    ]
    res = bass_utils.run_bass_kernel_spmd(
        nc,
        in_maps,
        core_ids=list(range(N_CORES)),
        trace=_trace,
        tmpdir=_tmpdir,
    )
    outs = [res.results[c]["out"] for c in range(N_CORES)]
    full = np.concatenate(outs, axis=0).reshape(4, 4096, DIM).astype(np.float32)
    if _trace:
        return full, res
    return full


if __name__ == "__main__":
    nc = bass.Bass()
    build(nc, 6)
    nc.finalize()
    print("build+compile OK")


# revision 16
# speedup vs baseline: 1.0889x; 1.0889x over previous
"""BitFeedForward (BitNet b1.58 MLP) Trainium2 kernel, v2.

Full computation:
    h = gelu(bitlinear(x, w1, b1));  out = bitlinear(h, w2, b2)
    bitlinear(x,w,b) = actquant(rmsnorm(x)) @ ternary(w).T + b

Sharding: pure data-parallel over the 16384 tokens -> 2048 tokens/core on
8 NeuronCores.  No collectives.  Weights arrive host-pre-transposed
([K, out] layout; pure data movement) and are quantized on device.

Key numerics (same as v1):
  - rmsnorm factor cancels inside activation_quant's round(); the rsqrt
    chain only feeds the post-matmul dequant scale gamma = amax*r*mean|w|/127
  - quantized activations / ternary weights are exact in bf16; matmuls
    run bf16 x bf16 with f32 PSUM accumulation -> integer-exact
  - round() = fl(c*x + 1.5*2^23) - 1.5*2^23 (RNE); ACT table stays on the
    gelu set (Copy/Abs/Gelu only)

v2 structural changes vs v1 (802us -> target ~560us):
  - two-stream weight phase: w1/w2 are each read twice from HBM (stats
    stream, then quant stream) instead of being staged in a 128KB SBUF
    pool.  The quant stream is c-major so mm1 of tile 0 can start after
    the first 8 chunks: PE starts at ~55us instead of ~180us.
  - per-token biases are folded into the matmuls as a K=1 accumulation
    row (stationary = 1/gamma per token, moving = bias row), so the PSUM
    dequant is a single ACT op: h = Gelu(psum * gamma) for mm1 and
    out = Copy(psum * gamma2) for mm2.  No DVE pass over PSUM at all;
    the psum bank is recycled by exactly one engine (ACT), which removes
    the mm1-start stalls the v1 trace showed on S[165].
  - amax(h) via a tensor_max pair-tree (gelu output is >= -0.17, so the
    row max equals the abs-max with probability 1 up to 2^-4096) instead
    of eight 700ns DVE abs-max reduces.
  - k2 rounding moved to the idle GpSimd engine; x/out DMAs moved off
    the Sync queue (x on SP only in A; out on gpsimd) so SP carries only
    the dma transposes.
  - 1/gamma rows ([1,128] stationaries) produced by a tiny DRAM
    round-trip with an f32->bf16 cast on the SWDGE leg, issued ~3 tiles
    ahead of use.
  - HAM warmup/filler matmuls keep the PE clock at 8/8 through the
    w1-chase gaps of tile 0.
"""

import sys

for _p in ("/opt/trn_rl_repo",):
    if _p not in sys.path:
        sys.path.insert(0, _p)

from contextlib import ExitStack

import numpy as np

import concourse.bass as bass
import concourse.mybir as mybir
import concourse.tile as tile
from concourse.bass import ts

F32 = mybir.dt.float32
BF16 = mybir.dt.bfloat16
AF = mybir.ActivationFunctionType
ALU = mybir.AluOpType
AX = mybir.AxisListType.X
AXY = mybir.AxisListType.XY

P = 128
DIM = 1024
INNER = 4096
N_CORES = 8
TOKENS = 4 * 4096
TOK_PER_CORE = TOKENS // N_CORES  # 2048
TT = TOK_PER_CORE // P  # 16 token tiles per core
CC = 1024
LAG = 2  # mm2(t-LAG) interleaves with mm1(t)

MMAGIC = 12582912.0  # 1.5 * 2**23 : RNE rounding magic for |x| < 2^22
QB = 127.0
EPS = 1e-5

_DONE = object()


def _split_dma_waits(nc):
    """walrus codegen only supports ONE sync wait on DMA pseudo-instructions.
    Move all but one wait onto standalone EventSemaphore (add 0) instructions
    on the issuing engine -- semantically identical, codegen-legal."""
    idc = 0
    for f in nc.m.functions:
        for bb in f.blocks:
            changed = False
            new = []
            for inst in bb.instructions:
                tn = type(inst).__name__
                si = inst.sync_info
                if (
                    tn != "InstEventSemaphore"
                    and si is not None
                    and len(si.on_wait) > 1
                ):
                    waits = list(si.on_wait)
                    for w in waits[:-1]:
                        idc += 1
                        e = mybir.InstEventSemaphore(
                            name=f"WSPLIT-{idc}",
                            sync_type="semaphore",
                            id=w.id,
                            update_mode="sem-add-imm",
                            update_value=0,
                        )
                        e.engine = inst.engine
                        e.sync_info = mybir.SyncInfo(on_wait=[w], on_update=[])
                        nc.register_instruction(e, overwrite=True)
                        new.append(e)
                    inst.sync_info = mybir.SyncInfo(
                        on_wait=[waits[-1]], on_update=list(si.on_update)
                    )
                    changed = True
                new.append(inst)
            if changed:
                bb.instructions = new


def build(nc: bass.Bass, n_ttiles: int = TT, af_act=None):
    if af_act is None:
        af_act = AF.Gelu
    toks = n_ttiles * P
    lag = min(LAG, max(1, n_ttiles - 1))
    x_d = nc.dram_tensor("x", [toks, DIM], F32, kind="ExternalInput")
    # weights arrive pre-transposed from the host: [K, out]
    w1_d = nc.dram_tensor("w1t", [DIM, INNER], F32, kind="ExternalInput")
    b1_d = nc.dram_tensor("b1", [INNER], F32, kind="ExternalInput")
    w2_d = nc.dram_tensor("w2t", [INNER, DIM], F32, kind="ExternalInput")
    b2_d = nc.dram_tensor("b2", [DIM], F32, kind="ExternalInput")
    out_d = nc.dram_tensor("out", [toks, DIM], F32, kind="ExternalOutput")

    with tile.TileContext(nc) as tc, ExitStack() as ctx:
        consts = ctx.enter_context(tc.tile_pool(name="consts", bufs=1))
        dram = ctx.enter_context(tc.tile_pool(name="dram", bufs=1, space="DRAM"))
        gdram = ctx.enter_context(tc.tile_pool(name="gdram", bufs=6, space="DRAM"))
        psA = ctx.enter_context(tc.tile_pool(name="psA", bufs=5, space="PSUM"))
        psB = ctx.enter_context(tc.tile_pool(name="psB", bufs=2, space="PSUM"))
        psS = ctx.enter_context(tc.tile_pool(name="psS", bufs=1, space="PSUM"))
        st_p = ctx.enter_context(tc.tile_pool(name="st", bufs=2))
        g_p = ctx.enter_context(tc.tile_pool(name="g", bufs=lag + 1 if lag < 2 else 3))
        gr_p = ctx.enter_context(tc.tile_pool(name="gr", bufs=2))

        ones = consts.tile([P, 1], F32)
        nc.vector.memset(ones, 1.0)
        ones_bf = consts.tile([P, 1], BF16)
        nc.vector.memset(ones_bf, 1.0)
        # w1Tc[c][:, j, m]: K-slice j (128 rows), inner cols c*1024 + m
        w1Tc = [
            consts.tile([P, 8, CC], BF16, tag=f"w1T{c}", name=f"w1T{c}")
            for c in range(4)
        ]
        # w2T[:, r, m]: K-slice r of 32 (128 rows of w2t), out m (1024)
        w2T = consts.tile([P, 32, DIM], BF16, tag="w2T")
        # bcast scalars: 0=ws1 1=mwd1 2=ws2 3=mwd2 4=127*ws1 5=127*ws2
        scal = consts.tile([P, 6], F32, tag="scal")
        b1row = consts.tile([1, INNER], BF16, tag="b1row")
        b2row = consts.tile([1, DIM], BF16, tag="b2row")
        dsc = dram.tile([1, 6], F32)

        wstg_p = ctx.enter_context(tc.tile_pool(name="wstg", bufs=4))
        xin_p = ctx.enter_context(tc.tile_pool(name="xin", bufs=1))
        k1q_p = ctx.enter_context(tc.tile_pool(name="k1q", bufs=1))
        k1T_p = ctx.enter_context(tc.tile_pool(name="k1T", bufs=2))
        k2m_p = ctx.enter_context(tc.tile_pool(name="k2m", bufs=1))
        qb_p = ctx.enter_context(tc.tile_pool(name="qb", bufs=1))
        k2T_p = ctx.enter_context(tc.tile_pool(name="k2T", bufs=lag + 1))
        h_p = ctx.enter_context(tc.tile_pool(name="h", bufs=4))
        out_p = ctx.enter_context(tc.tile_pool(name="out", bufs=1))

        # ---------------- helpers ----------------
        def _bcast(dram_sc, sb_dst, src):
            nc.sync.dma_start(dram_sc, src)
            nc.sync.dma_start(sb_dst, dram_sc.to_broadcast(list(sb_dst.shape)))

        def _rsqrt_newton(v, seed, iters, tg):
            r = st_p.tile([P, 1], F32, tag=f"rs_r{tg}")
            nc.vector.memset(r, seed)
            for _ in range(iters):
                rr = st_p.tile([P, 1], F32, tag=f"rs_rr{tg}")
                nc.vector.tensor_mul(rr, r, r)
                t = st_p.tile([P, 1], F32, tag=f"rs_t{tg}")
                nc.vector.scalar_tensor_tensor(
                    out=t, in0=rr, scalar=-0.5, in1=v, op0=ALU.mult, op1=ALU.mult
                )
                r2 = st_p.tile([P, 1], F32, tag=f"rs_r2{tg}")
                nc.vector.scalar_tensor_tensor(
                    out=r2, in0=t, scalar=1.5, in1=r, op0=ALU.add, op1=ALU.mult
                )
                r = r2
            return r

        def _chain_pre(amax, mv, seed, iters, tg):
            """c = 127/max(amax,eps); plus am, rec, r=rsqrt(var+mean^2+eps),
            sq=sqrt(var+mean^2+eps) for the scal-dependent tail."""
            am = g_p.tile([P, 1], F32, tag=f"c_am{tg}")
            nc.vector.tensor_scalar(
                out=am, in0=amax, scalar1=EPS, scalar2=None, op0=ALU.max
            )
            rec = g_p.tile([P, 1], F32, tag=f"c_rec{tg}")
            nc.vector.reciprocal(rec, am)
            c = st_p.tile([P, 1], F32, tag=f"c_c{tg}")
            nc.vector.tensor_scalar(
                out=c, in0=rec, scalar1=QB, scalar2=None, op0=ALU.mult
            )
            v = st_p.tile([P, 1], F32, tag=f"g_v{tg}")
            nc.vector.tensor_scalar(
                out=v, in0=mv[:, 0:1], scalar1=mv[:, 0:1], scalar2=None, op0=ALU.mult
            )
            nc.vector.tensor_scalar(
                out=v, in0=v, scalar1=mv[:, 1:2], scalar2=EPS, op0=ALU.add, op1=ALU.add
            )
            r = _rsqrt_newton(v, seed, iters, tg)
            rk = g_p.tile([P, 1], F32, tag=f"g_r{tg}")
            nc.vector.tensor_copy(rk, r)
            sq = g_p.tile([P, 1], F32, tag=f"g_sq{tg}")
            nc.vector.tensor_mul(sq, v, r)  # sqrt(v) = v * rsqrt(v)
            return c, (am, rec, rk, sq)

        def _chain_post(pre, mwd_col, qws_col, tg):
            """g = am*mwd*r ; ginv = rec*qws*sq ; row = [1,P] bf16 of ginv"""
            am, rec, r, sq = pre
            g = g_p.tile([P, 1], F32, tag=f"g{tg}")
            nc.vector.scalar_tensor_tensor(
                out=g,
                in0=am,
                scalar=scal[:, mwd_col : mwd_col + 1],
                in1=r,
                op0=ALU.mult,
                op1=ALU.mult,
            )
            ginv = st_p.tile([P, 1], F32, tag=f"gi{tg}")
            nc.vector.scalar_tensor_tensor(
                out=ginv,
                in0=rec,
                scalar=scal[:, qws_col : qws_col + 1],
                in1=sq,
                op0=ALU.mult,
                op1=ALU.mult,
            )
            row = _ginv_row(ginv, tg)
            return g, row

        def _ginv_row(ginv, tg):
            """[P,1] f32 -> [1,P] bf16 via DRAM round-trip (cast on SWDGE)."""
            gd = gdram.tile([P, 1], F32, tag=f"gd{tg}")
            nc.gpsimd.dma_start(gd, ginv)
            row = gr_p.tile([1, P], BF16, tag=f"grow{tg}")
            nc.gpsimd.dma_start(row, gd.rearrange("p o -> o p"))
            return row

        _PARTS = {}

        def wscale(tg, n_elems, ws_col, mwd_col, qws_col, dsc_off):
            partials = _PARTS[tg]
            psum_v = st_p.tile([P, 1], F32, tag=f"psumv{tg}")
            nc.vector.tensor_reduce(out=psum_v, in_=partials, axis=AX, op=ALU.add)
            tot = psS.tile([1, 1], F32, tag="tot")
            nc.tensor.matmul(tot, psum_v, ones[:, 0:1], start=True, stop=True)
            mean = st_p.tile([1, 1], F32, tag=f"mean{tg}")
            nc.scalar.activation(mean, tot, AF.Copy, bias=0.0, scale=1.0 / n_elems)
            mw = st_p.tile([1, 1], F32, tag=f"mw{tg}")
            nc.vector.tensor_scalar(
                out=mw, in0=mean, scalar1=EPS, scalar2=None, op0=ALU.max
            )
            wsv = st_p.tile([1, 1], F32, tag=f"wsv{tg}")
            nc.vector.reciprocal(wsv, mw)
            mwd = st_p.tile([1, 1], F32, tag=f"mwd{tg}")
            nc.vector.tensor_scalar(
                out=mwd, in0=mw, scalar1=1.0 / QB, scalar2=None, op0=ALU.mult
            )
            qws = st_p.tile([1, 1], F32, tag=f"qws{tg}")
            nc.vector.tensor_scalar(
                out=qws, in0=wsv, scalar1=QB, scalar2=None, op0=ALU.mult
            )
            _bcast(dsc[0:1, dsc_off : dsc_off + 1], scal[:, ws_col : ws_col + 1], wsv)
            _bcast(
                dsc[0:1, dsc_off + 1 : dsc_off + 2],
                scal[:, mwd_col : mwd_col + 1],
                mwd,
            )
            _bcast(
                dsc[0:1, dsc_off + 4 : dsc_off + 5],
                scal[:, qws_col : qws_col + 1],
                qws,
            )

        def quant_chunk(wf, dst, ws_b, parity):
            """round(wf*ws) -> dst (bf16, clipped to [-1,1]).  wf is consumed."""
            nc.gpsimd.tensor_scalar(
                out=wf,
                in0=wf,
                scalar1=ws_b,
                scalar2=MMAGIC,
                op0=ALU.mult,
                op1=ALU.add,
            )
            if parity == 0:
                nc.scalar.activation(dst, wf, AF.Copy, bias=-MMAGIC, scale=1.0)
            else:
                nc.vector.tensor_scalar(
                    out=dst, in0=wf, scalar1=MMAGIC, scalar2=None, op0=ALU.subtract
                )
            nc.vector.tensor_scalar(
                out=dst, in0=dst, scalar1=-1.0, scalar2=1.0, op0=ALU.max, op1=ALU.min
            )

        # ---------------- token-loop stages ----------------
        A = {}

        def emit_A(t):
            xin = xin_p.tile([P, DIM], F32, tag="x")
            nc.sync.dma_start(xin, x_d[ts(t, P), :])
            amax = st_p.tile([P, 1], F32, tag="amax")
            nc.vector.tensor_reduce(
                out=amax, in_=xin, axis=AX, op=ALU.max, apply_absolute_value=True
            )
            stat6 = st_p.tile([P, 2, 6], F32, tag="st6")
            xv = xin.rearrange("p (a b) -> p a b", a=2)
            for a in range(2):
                nc.vector.bn_stats(out=stat6[:, a, :], in_=xv[:, a, :])
            mv = st_p.tile([P, 2], F32, tag="mv")
            nc.vector.bn_aggr(out=mv, in_=stat6)
            c1, pre1 = _chain_pre(amax, mv, 1.0, 4, "1")
            k1 = k1q_p.tile([P, DIM], BF16, tag="k1q")
            for q in range(2):
                k1m = k2m_p.tile([P, 512], F32, tag="k2m", name=f"k1m{q}")
                nc.scalar.activation(
                    k1m, xin[:, ts(q, 512)], AF.Copy, bias=MMAGIC, scale=c1
                )
                nc.vector.tensor_scalar(
                    out=k1[:, ts(q, 512)],
                    in0=k1m,
                    scalar1=MMAGIC,
                    scalar2=None,
                    op0=ALU.subtract,
                )
            k1T = k1T_p.tile([P, 8, P], BF16, tag="k1T")
            nc.sync.dma_start_transpose(k1T, k1)
            A[t] = [k1T, pre1, None, None]

        def emit_A_post(t):
            g1, g1row = _chain_post(A[t][1], 1, 4, "1")
            A[t][2] = g1
            A[t][3] = g1row

        def emit_B_slice(t, n):
            k1T, _, g1, g1row = A[t]
            ps = psA.tile([P, 512], F32, tag="ps1")
            nc.tensor.matmul(
                ps, g1row, b1row[0:1, ts(n, 512)], start=True, stop=False
            )
            for j in range(8):
                nc.tensor.matmul(
                    ps,
                    k1T[:, j, :],
                    w1Tc[n // 2][:, j, (n % 2) * 512 : (n % 2) * 512 + 512],
                    start=False,
                    stop=(j == 7),
                )
            h = _HCH[t % 2][n // 2][:, (n % 2) * 512 : (n % 2) * 512 + 512]
            nc.scalar.activation(h, ps, af_act, bias=0.0, scale=g1)
            nc.vector.bn_stats(out=_HST[t % 2][:, n, :], in_=h)

        _HCH = {0: None, 1: None}
        _HST = {0: None, 1: None}

        def begin_B(t):
            _HCH[t % 2] = [
                h_p.tile([P, CC], BF16, tag="h", name=f"h{t}_{i}") for i in range(4)
            ]
            _HST[t % 2] = st_p.tile(
                [P, 8, 6], F32, tag=f"hst{t % 2}", name=f"hst{t}"
            )

        D = {}

        def emit_C(t):
            hch = _HCH[t % 2]
            m = k2m_p.tile([P, 512], BF16, tag="k2m", name="mtree")
            nc.vector.tensor_max(m, hch[0][:, 0:512], hch[0][:, 512:1024])
            for cc in range(1, 4):
                nc.vector.tensor_max(m, m, hch[cc][:, 0:512])
                nc.vector.tensor_max(m, m, hch[cc][:, 512:1024])
            amaxh = st_p.tile([P, 1], F32, tag="amaxh")
            nc.vector.tensor_reduce(out=amaxh, in_=m, axis=AX, op=ALU.max)
            mvh = st_p.tile([P, 2], F32, tag="mvh")
            nc.vector.bn_aggr(out=mvh, in_=_HST[t % 2])
            c2, pre2 = _chain_pre(amaxh, mvh, 1.75, 4, "2")
            k2T = k2T_p.tile([P, 4, 8, P], BF16, tag="k2T")
            for cc in range(4):
                k2c = qb_p.tile([P, CC], BF16, tag="qb", name=f"k2c{cc}")
                for q in range(2):
                    k2m = k2m_p.tile([P, 512], F32, tag="k2m", name=f"k2m{q}")
                    nc.gpsimd.tensor_scalar(
                        out=k2m,
                        in0=hch[cc][:, ts(q, 512)],
                        scalar1=c2,
                        scalar2=MMAGIC,
                        op0=ALU.mult,
                        op1=ALU.add,
                    )
                    nc.vector.tensor_scalar(
                        out=k2c[:, ts(q, 512)],
                        in0=k2m,
                        scalar1=MMAGIC,
                        scalar2=None,
                        op0=ALU.subtract,
                    )
                nc.sync.dma_start_transpose(k2T[:, cc, :, :], k2c)
            D[t] = [k2T, pre2, None, None]

        def emit_C_post(t):
            g2, g2row = _chain_post(D[t][1], 3, 5, "2")
            D[t][2] = g2
            D[t][3] = g2row

        def emit_D(t):
            k2T, _, g2, g2row = D.pop(t)
            for n in range(2):
                ps2 = psB.tile([P, 512], F32, tag="ps2")
                nc.tensor.matmul(
                    ps2, g2row, b2row[0:1, ts(n, 512)], start=True, stop=False
                )
                for cc in range(4):
                    for j in range(8):
                        nc.tensor.matmul(
                            ps2,
                            k2T[:, cc, j, :],
                            w2T[:, 8 * cc + j, ts(n, 512)],
                            start=False,
                            stop=(cc == 3 and j == 7),
                        )
                ot = out_p.tile([P, 512], F32, tag="ot", name=f"ot{n}")
                nc.scalar.activation(ot, ps2, AF.Copy, bias=0.0, scale=g2)
                nc.gpsimd.dma_start(
                    out_d[ts(t, P), ts(n, 512)], ot
                )

        def warm_pe(src, k):
            for _ in range(k):
                pw = psA.tile([P, 512], F32, tag="ps1")
                nc.tensor.matmul(pw[0:1, :], ones_bf, src, start=True, stop=True)

        # ---------------- W phase ----------------
        # w1 stats stream (HBM pass 1)
        part1 = st_p.tile([P, 32], F32, tag="partw1")
        _PARTS["w1"] = part1
        for c in range(4):
            for j in range(8):
                wf = wstg_p.tile([P, CC], F32, tag="wstg")
                nc.gpsimd.dma_start(wf, w1_d[ts(j, P), ts(c, CC)])
                nc.vector.tensor_reduce(
                    out=part1[:, c * 8 + j : c * 8 + j + 1],
                    in_=wf,
                    axis=AX,
                    op=ALU.add,
                    apply_absolute_value=True,
                )
        emit_A(0)
        emit_A(1)
        wscale("w1", 32 * P * 1024, 0, 1, 4, 0)
        ws1_b = scal[:, 0:1]
        emit_A_post(0)
        emit_A_post(1)
        # bias rows (f32 -> bf16 cast on the SWDGE leg)
        nc.gpsimd.dma_start(b1row, b1_d.rearrange("(a c) -> a c", a=1))
        nc.gpsimd.dma_start(b2row, b2_d.rearrange("(a c) -> a c", a=1))

        # w1 quant stream (HBM pass 2), c-major so mm1 chunks unlock in order
        begin_B(0)
        for c in range(4):
            for j in range(8):
                wf = wstg_p.tile([P, CC], F32, tag="wstg")
                nc.gpsimd.dma_start(wf, w1_d[ts(j, P), ts(c, CC)])
                quant_chunk(wf, w1Tc[c][:, j, :], ws1_b, j % 2)
            if c == 0:
                warm_pe(w1Tc[0][:, 0, 0:512], 12)
            emit_B_slice(0, 2 * c)
            emit_B_slice(0, 2 * c + 1)
            if c < 3:
                warm_pe(w1Tc[c][:, 7, 0:512], 8)
        emit_C(0)

        # w2: stats stream + scale + quant stream, interleaved into tiles 1..3
        def w2gen():
            part2 = st_p.tile([P, 32], F32, tag="partw2")
            _PARTS["w2"] = part2
            for r in range(32):
                wf = wstg_p.tile([P, CC], F32, tag="wstg")
                nc.scalar.dma_start(wf, w2_d[ts(r, P), :])
                if r % 2 == 0:
                    nc.vector.tensor_reduce(
                        out=part2[:, r : r + 1],
                        in_=wf,
                        axis=AX,
                        op=ALU.add,
                        apply_absolute_value=True,
                    )
                else:
                    nc.scalar.activation(
                        wf,
                        wf,
                        AF.Abs,
                        bias=0.0,
                        scale=1.0,
                        accum_out=part2[:, r : r + 1],
                    )
                yield
            wscale("w2", 32 * P * DIM, 2, 3, 5, 1)
            yield
            ws2_b = scal[:, 2:3]
            for r in range(32):
                wf = wstg_p.tile([P, CC], F32, tag="wstg")
                nc.scalar.dma_start(wf, w2_d[ts(r, P), :])
                quant_chunk(wf, w2T[:, r, :], ws2_b, r % 2)
                yield

        gens = {"g": w2gen(), "alive": True}

        def il_step(k=1):
            for _ in range(k):
                if gens["alive"]:
                    gens["alive"] = next(gens["g"], _DONE) is not _DONE

        # ---------------- token loop ----------------
        for t in range(1, n_ttiles):
            begin_B(t)
            for n in range(8):
                emit_B_slice(t, n)
                il_step(5)
            if t + 1 < n_ttiles:
                emit_A(t + 1)
                emit_A_post(t + 1)
            emit_C(t)
            if t == lag:
                while gens["alive"]:
                    il_step()
                for u in range(lag):
                    emit_C_post(u)
            elif t > lag:
                emit_C_post(t - 1)
            if t >= lag:
                emit_D(t - lag)
        while gens["alive"]:
            il_step()
        for u in range(n_ttiles):
            if u in D and D[u][2] is None:
                emit_C_post(u)
        for t in range(n_ttiles - lag, n_ttiles):
            emit_D(t)

    _split_dma_waits(nc)
    return nc, x_d, out_d


_CACHE = {}


def _get_compiled(n_ttiles=TT):
    if n_ttiles not in _CACHE:
        nc = bass.Bass()
        build(nc, n_ttiles)
        nc.finalize()
        _CACHE[n_ttiles] = nc
    return _CACHE[n_ttiles]


def kernel(x, w1, b1, w2, b2, _trace=False, _tmpdir=None):
    from concourse import bass_utils

    nc = _get_compiled(TT)
    xf = np.ascontiguousarray(x.reshape(TOKENS, DIM).astype(np.float32))
    w1t = np.ascontiguousarray(w1.astype(np.float32).T)  # [1024, 4096]
    b1 = np.ascontiguousarray(b1.astype(np.float32))
    w2t = np.ascontiguousarray(w2.astype(np.float32).T)  # [4096, 1024]
    b2 = np.ascontiguousarray(b2.astype(np.float32))
    in_maps = [
        {
            "x": xf[c * TOK_PER_CORE : (c + 1) * TOK_PER_CORE],
            "w1t": w1t,
            "b1": b1,
            "w2t": w2t,
            "b2": b2,
        }
        for c in range(N_CORES)
    ]
    res = bass_utils.run_bass_kernel_spmd(
        nc,
        in_maps,
        core_ids=list(range(N_CORES)),
        trace=_trace,
        tmpdir=_tmpdir,
    )
    outs = [res.results[c]["out"] for c in range(N_CORES)]
    full = np.concatenate(outs, axis=0).reshape(4, 4096, DIM).astype(np.float32)
    if _trace:
        return full, res
    return full


if __name__ == "__main__":
    nc = bass.Bass()
    build(nc, 6)
    nc.finalize()
    print("build+compile OK")
